# revision 13
# baseline (speedup 1.0000x reference)
"""Trainium2 Bass kernel: 8-head MultiHeadAttention (B=4, N=2048, E=512).

Sharding: 8 cores = 4 batches x 2 query-halves (data parallel). Each core
computes K/V for its whole batch (keys ordered own-half-first, other-half
second -- softmax is invariant to key permutation as long as K and V agree),
attention for its 1024 queries x all 8 heads, and its slice of the output
projection. No collectives; the host concatenates the 8 [1024, 512] slices.

Device-side layout choices:
  - All projections produce feature-major tensors (K^T/Q^T: [head*64+d, tok])
    so attention scores are computed directly as S^T = K' @ Q^T (the 1/sqrt(64)
    scale is folded into wk/bk on the host).
  - exp(S^T) runs on the scalar engine straight out of PSUM.
  - V is produced token-major with a fused ones-column (V' = [V_h | 1]) so the
    attention-output matmul also yields the softmax denominators for free.
  - Normalization multiplies by reciprocals broadcast across partitions with a
    1-row PE matmul (ones[1,64]^T @ recip[1,q]).
  - Matmul operands are bit-cast to float32r (full PE rate; plain float32
    matmuls run at 1/4 rate).
"""

import os
import sys

import numpy as np

for _p in ("/opt/trn_rl_repo", "/root/.axon_site/_ro/trn_rl_repo"):
    if os.path.isdir(_p) and _p not in sys.path:
        sys.path.insert(0, _p)

import concourse.bass as bass
from concourse import bacc
import concourse.tile as tile
from concourse import mybir
from concourse.bass_utils import run_bass_kernel_spmd

P = 128          # partitions
E = 512          # embed dim
H = 8            # heads
DH = 64          # head dim
T = 2048         # tokens per batch
NQ = 1024        # queries per core
FC = 4           # contraction chunks (512 / 128)
EC = 4           # output-feature chunks
KC = 16          # key-token chunks (2048 / 128)
B = 4
N_CORES = 8

F32 = mybir.dt.float32
F32R = mybir.dt.float32r
ADD = mybir.AluOpType.add
MUL = mybir.AluOpType.mult
EXP = mybir.ActivationFunctionType.Exp


def build_nc():
    nc = bacc.Bacc(trn_type="TRN2")

    xq = nc.declare_dram_parameter("xq", [E, NQ], F32R, isOutput=False)
    xo = nc.declare_dram_parameter("xo", [E, NQ], F32R, isOutput=False)
    wqt = nc.declare_dram_parameter("wqt", [E, E], F32R, isOutput=False)
    wkt = nc.declare_dram_parameter("wkt", [E, E], F32R, isOutput=False)
    wvt = nc.declare_dram_parameter("wvt", [E, E], F32R, isOutput=False)
    wot = nc.declare_dram_parameter("wot", [E, E], F32R, isOutput=False)
    bqp = nc.declare_dram_parameter("bqp", [P, EC], F32, isOutput=False)
    bkp = nc.declare_dram_parameter("bkp", [P, EC], F32, isOutput=False)
    bvb = nc.declare_dram_parameter("bvb", [P, E], F32, isOutput=False)
    bob = nc.declare_dram_parameter("bob", [P, E], F32, isOutput=False)
    out = nc.declare_dram_parameter("out", [NQ, E], F32, isOutput=True)

    with tile.TileContext(nc) as tc:
        with (
            tc.tile_pool(name="const", bufs=1) as cp,
            tc.tile_pool(name="attn", bufs=1) as atp,
            tc.tile_pool(name="kq", bufs=2) as kqp,
            tc.tile_pool(name="vpool", bufs=1) as vpp,
            tc.tile_pool(name="pin", bufs=1) as pin,
            tc.tile_pool(name="exps", bufs=2) as xsp,
            tc.tile_pool(name="norm", bufs=2) as nrm,
            tc.tile_pool(name="osb", bufs=2) as osb,
            tc.tile_pool(name="psA", bufs=2, space="PSUM") as psA,
            tc.tile_pool(name="psO", bufs=4, space="PSUM") as psO,
        ):
            # ---------- input loads ----------
            wk_t, xq_t, xo_t, wq_t, wv_t, wo_t = [], [], [], [], [], []
            for f in range(FC):
                w = pin.tile([P, E], F32R, name=f"wk{f}", tag=f"wk{f}")
                nc.sync.dma_start(w, wkt[f * P:(f + 1) * P, :])
                wk_t.append(w)
            for f in range(FC):
                t_ = pin.tile([P, NQ], F32R, name=f"xq{f}", tag=f"xq{f}")
                nc.sync.dma_start(t_, xq[f * P:(f + 1) * P, :])
                xq_t.append(t_)
            for f in range(FC):
                t_ = pin.tile([P, NQ], F32R, name=f"xo{f}", tag=f"xo{f}")
                nc.sync.dma_start(t_, xo[f * P:(f + 1) * P, :])
                xo_t.append(t_)
            for f in range(FC):
                w = pin.tile([P, E], F32R, name=f"wq{f}", tag=f"wq{f}")
                nc.sync.dma_start(w, wqt[f * P:(f + 1) * P, :])
                wq_t.append(w)
            for f in range(FC):
                w = pin.tile([P, E], F32R, name=f"wv{f}", tag=f"wv{f}")
                nc.sync.dma_start(w, wvt[f * P:(f + 1) * P, :])
                wv_t.append(w)
            for f in range(FC):
                w = cp.tile([P, E], F32R, name=f"wo{f}", tag=f"wo{f}")
                nc.sync.dma_start(w, wot[f * P:(f + 1) * P, :])
                wo_t.append(w)
            bq_t = cp.tile([P, EC], F32, name="bq", tag="bq")
            nc.sync.dma_start(bq_t, bqp[:, :])
            bk_t = cp.tile([P, EC], F32, name="bk", tag="bk")
            nc.sync.dma_start(bk_t, bkp[:, :])
            bvb_t = cp.tile([P, E], F32, name="bvb", tag="bvb")
            nc.sync.dma_start(bvb_t, bvb[:, :])
            bob_t = cp.tile([P, E], F32, name="bob", tag="bob")
            nc.sync.dma_start(bob_t, bob[:, :])
            ones_f = cp.tile([P, DH], F32, name="onesf", tag="onesf")
            nc.vector.memset(ones_f, 1.0)
            ones_t = cp.tile([33, DH], F32R, name="ones", tag="ones")
            nc.vector.tensor_copy(out=ones_t, in_=ones_f[0:33, :])

            # ---------- persistent activation tiles ----------
            vp = [vpp.tile([P, H, DH + 1], F32R, name=f"vp{t}", tag=f"vp{t}")
                  for t in range(KC)]
            ctx = [atp.tile([P, NQ], F32R, name=f"ctx{j}", tag=f"ctx{j}")
                   for j in range(EC)]

            def xcat(f, c0, w):
                # token columns [c0, c0+w) of concat(xq, xo), feature chunk f
                if c0 + w <= NQ:
                    return xq_t[f][:, c0:c0 + w]
                return xo_t[f][:, c0 - NQ:c0 - NQ + w]

            def emit_kt(j):
                kt_j = kqp.tile([P, T], F32R, name=f"kt{j}", tag="kt")
                for tcp in range(4):
                    ps = psA.tile([P, NQ], F32, name=f"pk{j}_{tcp}", tag="psA")
                    for f in range(FC):
                        nc.tensor.matmul(
                            ps[:, :E],
                            (wk_t[f][:, j * P:(j + 1) * P]),
                            (xcat(f, tcp * E, E)),
                            start=(f == 0), stop=(f == FC - 1),
                        )
                    nc.vector.tensor_scalar_add(
                        kt_j[:, tcp * E:(tcp + 1) * E], ps[:, :E], bk_t[:, j:j + 1])
                return kt_j

            def emit_qt(j):
                qt_j = kqp.tile([P, NQ], F32R, name=f"qt{j}", tag="qt")
                for tcp in range(2):
                    ps = psA.tile([P, NQ], F32, name=f"pq{j}_{tcp}", tag="psA")
                    for f in range(FC):
                        nc.tensor.matmul(
                            ps[:, :E],
                            (wq_t[f][:, j * P:(j + 1) * P]),
                            (xq_t[f][:, tcp * E:(tcp + 1) * E]),
                            start=(f == 0), stop=(f == FC - 1),
                        )
                    nc.vector.tensor_scalar_add(
                        qt_j[:, tcp * E:(tcp + 1) * E], ps[:, :E], bq_t[:, j:j + 1])
                return qt_j

            def emit_vp(t):
                ps = psA.tile([P, NQ], F32, name=f"pv{t}", tag="psA")
                for f in range(FC):
                    nc.tensor.matmul(
                        ps[:, :E],
                        (xcat(f, t * P, P)),
                        (wv_t[f]),
                        start=(f == 0), stop=(f == FC - 1),
                    )
                nc.vector.tensor_tensor(
                    vp[t][:, :, 0:DH],
                    ps[:, :E].rearrange("p (h d) -> p h d", d=DH),
                    bvb_t.rearrange("p (h d) -> p h d", d=DH),
                    ADD,
                )
                nc.vector.tensor_copy(
                    out=vp[t][:, :, DH:DH + 1], in_=ones_f[:, 0:H, None])

            def emit_head(h, kt_j, qt_j, sh):
                j, par = h // 2, h % 2
                o0 = psO.tile([DH + 1, E], F32, name=f"o0_{h}", tag="psO")
                o1 = psO.tile([DH + 1, E], F32, name=f"o1_{h}", tag="psO")
                outs = (o0, o1)
                r0, r1 = par * DH, (par + 1) * DH
                for k in range(KC):
                    if h == 0:
                        emit_vp(k)
                    s = psA.tile([P, NQ], F32, name=f"s{h}_{k}", tag="psA")
                    for qc in range(2):
                        nc.tensor.matmul(
                            s[:, qc * E:(qc + 1) * E],
                            (kt_j[r0:r1, k * P:(k + 1) * P]),
                            (qt_j[r0:r1, qc * E:(qc + 1) * E]),
                            start=True, stop=True,
                            tile_position=(par * DH, 0),
                        )
                    ex = xsp.tile([P, NQ], F32R, name=f"ex{h}_{k}", tag="ex")
                    nc.scalar.activation(ex, s, EXP)
                    for qc in range(2):
                        nc.tensor.matmul(
                            outs[qc],
                            (vp[k][:, h, :]),
                            (ex[:, qc * E:(qc + 1) * E]),
                            start=(k == 0), stop=(k == KC - 1),
                        )
                for qc, o in enumerate(outs):
                    # softmax denominators (ones-column row of o) -> row 32*par
                    nc.vector.tensor_copy(
                        out=sh[32 * par:32 * par + 1, qc * E:(qc + 1) * E],
                        in_=o[DH:DH + 1, :])
                    nc.vector.tensor_copy(
                        out=ctx[j][r0:r1, qc * E:(qc + 1) * E], in_=o[0:DH, :])

            def emit_normalize(j, sh):
                # reciprocal of this pair's softmax denominators (rows 0/32),
                # broadcast across the 64 head-dim partitions via a K=1 matmul.
                rp = nrm.tile([33, NQ], F32R, name=f"rp{j}", tag="rp")
                with nc.allow_low_precision(reason="f32r rounding of softmax denominators"):
                    nc.vector.reciprocal(rp, sh)
                for par in range(2):
                    rb = psA.tile([P, NQ], F32, name=f"rb{2 * j + par}", tag="psA")
                    for qc in range(2):
                        nc.tensor.matmul(
                            rb[0:DH, qc * E:(qc + 1) * E],
                            (ones_t[32 * par:32 * par + 1, :]),
                            (rp[32 * par:32 * par + 1, qc * E:(qc + 1) * E]),
                            start=True, stop=True,
                        )
                    rows = ctx[j][par * DH:(par + 1) * DH, :]
                    nc.vector.tensor_tensor(rows, rows, rb[0:DH, :], MUL)

            # ---------- projections + attention, interleaved ----------
            for j in range(EC):
                kt_j = emit_kt(j)
                qt_j = emit_qt(j)
                sh = nrm.tile([33, NQ], F32, name=f"sh{j}", tag="sh")
                nc.vector.memset(sh, 1.0)
                emit_head(2 * j, kt_j, qt_j, sh)
                emit_head(2 * j + 1, kt_j, qt_j, sh)
                emit_normalize(j, sh)

            # ---------- output projection ----------
            for qt_i in range(NQ // P):
                ps = psA.tile([P, NQ], F32, name=f"pf{qt_i}", tag="psA")
                for j in range(EC):
                    nc.tensor.matmul(
                        ps[:, :E],
                        (ctx[j][:, qt_i * P:(qt_i + 1) * P]),
                        (wo_t[j]),
                        start=(j == 0), stop=(j == EC - 1),
                    )
                ot = osb.tile([P, E], F32, name=f"ot{qt_i}", tag="ot")
                nc.vector.tensor_tensor(ot, ps[:, :E], bob_t, ADD)
                nc.sync.dma_start(out[qt_i * P:(qt_i + 1) * P, :], ot)

    nc.compile()
    return nc


_NC = None


def _get_nc():
    global _NC
    if _NC is None:
        _NC = build_nc()
    return _NC


def make_in_maps(q, wq, bq, wk, bk, wv, bv, wo, bo):
    q = np.asarray(q, np.float32)
    scale = 1.0 / np.sqrt(np.float32(DH))
    shared = dict(
        wqt=np.ascontiguousarray(np.asarray(wq, np.float32).T),
        wkt=np.ascontiguousarray(np.asarray(wk, np.float32).T * scale),
        wvt=np.ascontiguousarray(np.asarray(wv, np.float32).T),
        wot=np.ascontiguousarray(np.asarray(wo, np.float32).T),
        bqp=np.ascontiguousarray(np.asarray(bq, np.float32).reshape(EC, P).T),
        bkp=np.ascontiguousarray(
            (np.asarray(bk, np.float32) * scale).reshape(EC, P).T),
        bvb=np.ascontiguousarray(
            np.broadcast_to(np.asarray(bv, np.float32), (P, E))),
        bob=np.ascontiguousarray(
            np.broadcast_to(np.asarray(bo, np.float32), (P, E))),
    )
    in_maps = []
    for c in range(N_CORES):
        b, half = c // 2, c % 2
        xT = q[b].T
        in_maps.append(dict(
            xq=np.ascontiguousarray(xT[:, half * NQ:(half + 1) * NQ]),
            xo=np.ascontiguousarray(xT[:, (1 - half) * NQ:(2 - half) * NQ]),
            **shared,
        ))
    return in_maps


def assemble(results):
    full = np.empty((B, T, E), np.float32)
    for c in range(N_CORES):
        b, half = c // 2, c % 2
        full[b, half * NQ:(half + 1) * NQ, :] = results[c]["out"]
    return full


def kernel(q, wq, bq, wk, bk, wv, bv, wo, bo):
    in_maps = make_in_maps(q, wq, bq, wk, bk, wv, bv, wo, bo)
    nc = _get_nc()
    res = run_bass_kernel_spmd(nc, in_maps, list(range(N_CORES)))
    return assemble(res.results)
